# revision 1
# baseline (speedup 1.0000x reference)
"""Trainium2 Bass kernel for nn_Denoiser (GIN-VAE encoder + GAT decoder GNN).

Distribution strategy (8 NeuronCores, SPMD single NEFF):
  - Nodes are sharded by destination ownership: core k owns nodes
    [k*2500, (k+1)*2500). Edges (with self-loops appended) are sorted by dst
    and sharded to the core owning the dst.
  - Edge aggregation (segment-sum / segment-softmax) uses one batched
    dma_gather per 128-dst block (amortizes the ~1us SWDGE launch that
    dominated the per-chunk indirect-DMA version), followed by one-hot
    "selection" matmuls on the PE.
  - GAT runs dst-major: pm_h = se_h.T @ [hh_h | 1]; the ones column makes
    the softmax denominator fall out of the same accumulating matmul, and
    the per-dst normalization becomes a per-partition activation scale.
  - a_dst lookups are always core-local (dst ownership), so they gather
    from a local [own, 128] table with local int16 indices - no AllGather.
  - Dense per-node math is feature-major ([C partitions, nodes free]) and
    computed own-only; full tables needed for the next layer's gathers are
    materialized row-major in DRAM via PE transposes + AllGather (Shared
    address space for HBM-HBM collectives).
  - BatchNorm (training mode, global batch stats) uses a tiny AllReduce of
    per-feature partial sums.

Host-side prep: edge sort/shard/pad, int16 16-partition-wrapped gather
indices, transposed bf16/f32 inputs, small weight reshapes.
"""

import os
import sys

for _p in ("/opt/trn_rl_repo", "/root/.axon_site/_ro/trn_rl_repo"):
    if os.path.isdir(_p) and _p not in sys.path:
        sys.path.insert(0, _p)

from dataclasses import dataclass

import ml_dtypes
import numpy as np

import concourse.bacc as bacc
import concourse.bass as bass
import concourse.mybir as mybir
import concourse.tile as tile
from concourse.bass import AP
from concourse.bass_utils import run_bass_kernel_spmd

F32 = mybir.dt.float32
BF16 = mybir.dt.bfloat16
I16 = mybir.dt.int16
NPBF = ml_dtypes.bfloat16

BN_EPS = 1e-5


@dataclass
class Cfg:
    n: int = 20000        # total nodes
    ncores: int = 8
    din: int = 92
    c: int = 128          # hidden dim
    h: int = 2            # GAT heads
    cpb: int = 35         # chunks (of 128 edge slots) per dst block
    ts: int = 500         # dense free-dim tile size for own-node matmuls

    @property
    def own(self):
        return self.n // self.ncores

    @property
    def nblk(self):
        return -(-self.own // 128)

    @property
    def slots(self):
        return self.nblk * self.cpb * 128

    @property
    def bigw(self):
        # [hh0 | 1 | hh1 | 1 | a_s0 a_s1 | pad]; dma_gather rows must be a
        # multiple of 256 bytes -> 384 bf16.
        return 384


def _bcast_free(ap: AP, count: int) -> AP:
    """[P,1...] AP -> [P,count] AP with free step 0 (free-dim broadcast)."""
    rows = [list(r) for r in ap.ap]
    part = rows[0]
    assert all(r[1] == 1 for r in rows[1:]), rows
    return AP(ap.tensor, ap.offset, [part, [0, count]])


def build_program(cfg: Cfg, phase_limit: int = 99) -> bass.Bass:
    nc = bacc.Bacc(
        "TRN2",
        target_bir_lowering=False,
        debug=False,
        enable_asserts=False,
        num_devices=cfg.ncores,
    )
    n, own, nblk, cpb, ts = cfg.n, cfg.own, cfg.nblk, cfg.cpb, cfg.ts
    C, H, din = cfg.c, cfg.h, cfg.din
    slots = cfg.slots
    bigw = cfg.bigw
    groups = [list(range(cfg.ncores))]
    ntile_own = [min(128, own - t * 128) for t in range(nblk)]  # 128.. tail
    dts = [(i * ts, min(ts, own - i * ts)) for i in range(-(-own // ts))]
    # full-N dense tiles for the embedding phase
    fts = 512
    ftiles = [(i * fts, min(fts, n - i * fts)) for i in range(-(-n // fts))]
    shared_cc = bool(int(os.environ.get("KERNEL_SHARED_CC", "1")))
    cc_space = "Shared" if shared_cc else "Local"

    # ---------------- I/O ----------------
    di = {}  # dram inputs

    def inp(name, shape, dt):
        di[name] = nc.dram_tensor(name, list(shape), dt, kind="ExternalInput")
        return di[name]

    inp("nfT", [din, n], BF16)              # node features, transposed bf16
    inp("epsT", [128, own], F32)            # own eps slice, transposed
    inp("w_emb", [din, C], BF16)
    inp("w1", [C, C], BF16)
    inp("w2", [C, C], BF16)
    inp("w_mu", [C, C], BF16)
    inp("w_var", [C, C], BF16)
    inp("w_gat", [C, H * C], BF16)
    inp("w_dec0", [C, C], BF16)             # W_dec rows 0:128
    inp("w_dec1", [C, C], BF16)             # W_dec rows 128:256
    inp("attp0", [C, 2], BF16)              # [att_src[0] | att_dst[0]]
    inp("attp1", [C, 2], BF16)
    inp("bn_emb", [128, 2], F32)            # gamma | beta columns
    inp("bn_gin", [128, 2], F32)
    inp("b2c", [128, 1], F32)
    inp("bmuc", [128, 1], F32)
    inp("bvarc", [128, 1], F32)
    inp("bdecc", [128, 1], F32)             # b_gat @ W_dec + b_dec
    inp("idx_src", [128, slots // 16], I16)   # wrapped per 16p, replicated x8
    inp("idx_dstl", [128, slots // 16], I16)  # local dst ids, same wrap
    inp("dstw", [128, slots // 128], F32)   # block-local dst ids, -1 pad
    inp("iota", [128, 128], BF16)           # row = 0..127 on every partition
    inp("id_bf", [128, 128], BF16)
    inp("id_f32", [128, 128], F32)
    inp("ones_col_bf", [128, 1], BF16)

    outs = {}
    for nm in ("zin_s", "zout_s", "mu_s", "logvar_s"):
        outs[nm] = nc.dram_tensor(nm, [own, C], F32, kind="ExternalOutput")

    with tile.TileContext(nc) as tc:
        with (
            tc.tile_pool(name="state", bufs=1) as st,
            tc.tile_pool(name="dram", bufs=1, space="DRAM") as dr,
            tc.tile_pool(name="psum_d", bufs=2, space="PSUM") as psd,
            tc.tile_pool(name="psum_t", bufs=2, space="PSUM") as pst,
            tc.tile_pool(name="work", bufs=2) as wk,
        ):
            # ---------- load constants / inputs into SBUF ----------
            def load(name, shape, dt, pool=None):
                t = (pool or st).tile(shape, dt, tag=name, name=name)
                nc.sync.dma_start(t[:], di[name][:])
                return t

            QCH = 8  # chunks per dma_gather call (1024-descriptor ring limit)

            def gather_block(gt, table, idx_tile, b, elem):
                """Gather one dst block's rows in <=QCH-chunk dma_gather calls."""
                for q0 in range(0, cpb, QCH):
                    nq = min(QCH, cpb - q0)
                    c0 = (b * cpb + q0) * 8
                    nc.gpsimd.dma_gather(
                        gt[:, q0 : q0 + nq, :], table[:, :],
                        idx_tile[:, c0 : c0 + nq * 8],
                        nq * 128, nq * 128, elem,
                    )

            epsT = load("epsT", [128, own], F32)
            w_emb = load("w_emb", [din, C], BF16)
            w1 = load("w1", [C, C], BF16)
            w2 = load("w2", [C, C], BF16)
            w_mu = load("w_mu", [C, C], BF16)
            w_var = load("w_var", [C, C], BF16)
            w_gat = load("w_gat", [C, H * C], BF16)
            w_dec0 = load("w_dec0", [C, C], BF16)
            w_dec1 = load("w_dec1", [C, C], BF16)
            attp = [load("attp0", [C, 2], BF16), load("attp1", [C, 2], BF16)]
            bn_emb = load("bn_emb", [128, 2], F32)
            bn_gin = load("bn_gin", [128, 2], F32)
            b2c = load("b2c", [128, 1], F32)
            bmuc = load("bmuc", [128, 1], F32)
            bvarc = load("bvarc", [128, 1], F32)
            bdecc = load("bdecc", [128, 1], F32)
            idx_src = load("idx_src", [128, slots // 16], I16)
            idx_dstl = load("idx_dstl", [128, slots // 16], I16)
            dstw = load("dstw", [128, slots // 128], F32)
            iota = load("iota", [128, 128], BF16)
            id_bf = load("id_bf", [128, 128], BF16)
            id_f32 = load("id_f32", [128, 128], F32)
            ones_col_bf = load("ones_col_bf", [128, 1], BF16)

            # persistent DRAM tables
            x_table = [
                dr.tile([n, C], BF16, tag=f"x_table{i}", name=f"x_table{i}",
                        addr_space=(cc_space if i == 1 else "Local"))
                for i in range(2)
            ]
            big_table = [
                dr.tile([n, bigw], BF16, tag=f"big_table{i}",
                        name=f"big_table{i}", addr_space=cc_space)
                for i in range(2)
            ]
            ad_local = [
                dr.tile([own, 128], BF16, tag=f"ad_local{i}",
                        name=f"ad_local{i}")
                for i in range(2)
            ]
            cc_rows_big = dr.tile([own, bigw], BF16, tag="cc_rows_big",
                                  name="cc_rows_big")
            cc_rows_x = dr.tile([own, C], BF16, tag="cc_rows_x",
                                name="cc_rows_x")
            cc_stat_in = dr.tile([128, 2], F32, tag="cc_stat_in",
                                 name="cc_stat_in")
            cc_stat_out = [
                dr.tile([128, 2], F32, tag=f"cc_stat_out{i}",
                        name=f"cc_stat_out{i}")
                for i in range(2)
            ]

            # ---------- helpers ----------
            def bn_cols_from_stats(ssum, ssq, gamma_beta, count):
                """Return (k, b) [128,1] f32 columns: y -> y*k + b."""
                mean = wk.tile([128, 1], F32, tag="bn_mean", name="bn_mean")
                nc.vector.tensor_scalar_mul(mean[:], ssum, 1.0 / count)
                ex2 = wk.tile([128, 1], F32, tag="bn_ex2", name="bn_ex2")
                nc.vector.tensor_scalar_mul(ex2[:], ssq, 1.0 / count)
                m2 = wk.tile([128, 1], F32, tag="bn_m2", name="bn_m2")
                nc.vector.tensor_mul(m2[:], mean[:], mean[:])
                var = wk.tile([128, 1], F32, tag="bn_var", name="bn_var")
                nc.vector.tensor_sub(var[:], ex2[:], m2[:])
                nc.vector.tensor_scalar_add(var[:], var[:], BN_EPS)
                inv = wk.tile([128, 1], F32, tag="bn_inv", name="bn_inv")
                nc.vector.reciprocal(inv[:], var[:])
                rs = wk.tile([128, 1], F32, tag="bn_rs", name="bn_rs")
                nc.scalar.sqrt(rs[:], inv[:])
                k = wk.tile([128, 1], F32, tag="bn_k", name="bn_k")
                nc.vector.tensor_mul(k[:], rs[:], gamma_beta[:, 0:1])
                mk = wk.tile([128, 1], F32, tag="bn_mk", name="bn_mk")
                nc.vector.tensor_mul(mk[:], mean[:], k[:])
                b = wk.tile([128, 1], F32, tag="bn_b", name="bn_b")
                nc.vector.tensor_sub(b[:], gamma_beta[:, 1:2], mk[:])
                return k, b

            def stats_of(ytile, width):
                """Local per-feature sum and sum-of-squares of y [128,width]."""
                ssum = wk.tile([128, 1], F32, tag="st_ssum", name="st_ssum")
                nc.vector.tensor_reduce(
                    ssum[:], ytile[:, 0:width], axis=mybir.AxisListType.X,
                    op=mybir.AluOpType.add,
                )
                sq = wk.tile([128, len(dts)], F32, tag="st_sq", name="st_sq")
                for i, (o0, w_) in enumerate(dts):
                    scr = wk.tile([128, fts], BF16, tag="scr0", name="scr0")
                    nc.scalar.activation(
                        scr[:, 0:w_], ytile[:, o0 : o0 + w_],
                        mybir.ActivationFunctionType.Square,
                        accum_out=sq[:, i : i + 1],
                    )
                ssq = wk.tile([128, 1], F32, tag="st_ssq", name="st_ssq")
                nc.vector.tensor_reduce(
                    ssq[:], sq[:], axis=mybir.AxisListType.X, op=mybir.AluOpType.add
                )
                return ssum, ssq

            GW = 8  # node tiles per batched row-write DMA

            def write_rows(cols, dst_dram, width, src_dt, grp=GW,
                           fill_pad=False, const_cols=()):
                """Transpose f-major own tiles into row-major dst_dram [own,width].

                cols: list of (col_off, ncol, get_ap(t, nt) -> [ncol-part, nt] AP).
                const_cols: list of (col_off, [128,1] const tile) copied as-is.
                Full 128-node tiles are batched grp-at-a-time into one DMA; the
                tail tile (own % 128) gets its own DMA.
                """
                full = own // 128
                tail = own % 128
                ident_t = id_bf if src_dt == BF16 else id_f32

                def rowbuf_tile():
                    rb = wk.tile([128, grp * width], src_dt,
                                 tag=f"rows{width}_{src_dt}",
                                 name=f"rows{width}_{src_dt}")
                    if fill_pad:
                        nc.vector.memset(rb[:], 0.0)
                    return rb

                def do_tile(rowbuf, t, nt, g):
                    for (co, ncol, get) in cols:
                        src = get(t, nt)
                        pt = pst.tile([128, 128], src_dt, tag="t", name="pt")
                        nc.tensor.transpose(
                            pt[0:nt, 0:ncol], src, ident_t[0:ncol, 0:ncol]
                        )
                        nc.scalar.copy(
                            rowbuf[0:nt, g * width + co : g * width + co + ncol],
                            pt[0:nt, 0:ncol],
                        )
                    for (co, ctile) in const_cols:
                        nc.scalar.copy(
                            rowbuf[0:nt, g * width + co : g * width + co + 1],
                            ctile[0:nt, 0:1],
                        )

                t = 0
                while t < full:
                    gcnt = min(grp, full - t)
                    rowbuf = rowbuf_tile()
                    for g in range(gcnt):
                        do_tile(rowbuf, t + g, 128, g)
                    if gcnt == 1:
                        nc.sync.dma_start(
                            dst_dram[t * 128 : (t + 1) * 128, :],
                            rowbuf[:, 0:width],
                        )
                    else:
                        dst = dst_dram[t * 128 : (t + gcnt) * 128, :].rearrange(
                            "(g p) f -> p g f", p=128
                        )
                        nc.sync.dma_start(
                            dst,
                            rowbuf[:, 0 : gcnt * width].rearrange(
                                "p (g f) -> p g f", f=width
                            ),
                        )
                    t += gcnt
                if tail:
                    rowbuf = rowbuf_tile()
                    do_tile(rowbuf, full, tail, 0)
                    nc.sync.dma_start(
                        dst_dram[full * 128 : own, :], rowbuf[0:tail, 0:width]
                    )

            def dense_own(lhsTs, rhs_fn, out_tag, out_dt, pool, bias=None,
                          act=None, scale=None):
                """out[128, own] = act(scale * sum_i lhsTs[i].T @ rhs_i + bias)."""
                o = pool.tile([128, own], out_dt, tag=out_tag, name=out_tag)
                if act is not None:
                    func = act
                elif bias is None and scale is None:
                    func = mybir.ActivationFunctionType.Copy
                else:
                    func = mybir.ActivationFunctionType.Identity
                for (o0, w_) in dts:
                    pt = psd.tile([128, 512], F32, tag="d", name="pd")
                    for i, (lt, rf) in enumerate(zip(lhsTs, rhs_fn)):
                        nc.tensor.matmul(
                            pt[:, 0:w_], lt, rf(o0, w_),
                            start=(i == 0), stop=(i == len(lhsTs) - 1),
                        )
                    kw = {}
                    if bias is not None:
                        kw["bias"] = bias
                    if scale is not None:
                        kw["scale"] = scale
                    nc.scalar.activation(
                        o[:, o0 : o0 + w_], pt[:, 0:w_], func, **kw,
                    )
                return o

            # =================================================================
            # Phase 0: embedding (full N, redundant on every core)
            # =================================================================
            with tc.tile_pool(name="p0", bufs=1) as p0:
                y0 = p0.tile([128, n], BF16, tag="y0", name="y0")
                for (o0, w_) in ftiles:
                    nft = wk.tile([din, fts], BF16, tag="nft", name="nft")
                    nc.sync.dma_start(nft[:, 0:w_], di["nfT"][:, o0 : o0 + w_])
                    pt = psd.tile([128, 512], F32, tag="d", name="pd")
                    nc.tensor.matmul(
                        pt[:, 0:w_], w_emb[:], nft[:, 0:w_],
                        start=True, stop=True,
                    )
                    nc.scalar.copy(y0[:, o0 : o0 + w_], pt[:, 0:w_])
                ssum0 = wk.tile([128, 1], F32, tag="ssum0", name="ssum0")
                nc.vector.tensor_reduce(
                    ssum0[:], y0[:], axis=mybir.AxisListType.X,
                    op=mybir.AluOpType.add,
                )
                sqcols = wk.tile([128, len(ftiles)], F32, tag="sqcols", name="sqcols")
                scr0 = wk.tile([128, fts], BF16, tag="scr0", name="scr0")
                for i, (o0, w_) in enumerate(ftiles):
                    nc.scalar.activation(
                        scr0[:, 0:w_], y0[:, o0 : o0 + w_],
                        mybir.ActivationFunctionType.Square,
                        accum_out=sqcols[:, i : i + 1],
                    )
                ssq0 = wk.tile([128, 1], F32, tag="ssq0", name="ssq0")
                nc.vector.tensor_reduce(
                    ssq0[:], sqcols[:], axis=mybir.AxisListType.X,
                    op=mybir.AluOpType.add,
                )
                k0, b0 = bn_cols_from_stats(ssum0[:], ssq0[:], bn_emb, n)
                x0 = p0.tile([128, n], BF16, tag="x0", name="x0")
                nc.scalar.activation(
                    x0[:], y0[:], mybir.ActivationFunctionType.Relu,
                    bias=b0[:], scale=k0[:],
                )
                # write full x0 table (row-major, local; no collective needed)
                nfull = n // 128
                ntail = n % 128
                t = 0
                while t < nfull:
                    gcnt = min(GW, nfull - t)
                    rowbuf0 = wk.tile([128, GW * C], BF16, tag="rowbuf0",
                                      name="rowbuf0")
                    for g in range(gcnt):
                        pt = pst.tile([128, 128], BF16, tag="t", name="pt")
                        nc.tensor.transpose(
                            pt[:], x0[:, (t + g) * 128 : (t + g + 1) * 128],
                            id_bf[:],
                        )
                        nc.scalar.copy(rowbuf0[:, g * C : (g + 1) * C], pt[:])
                    if gcnt == 1:
                        nc.sync.dma_start(
                            x_table[0][t * 128 : (t + 1) * 128, :], rowbuf0[:, 0:C]
                        )
                    else:
                        dst = x_table[0][t * 128 : (t + gcnt) * 128, :].rearrange(
                            "(g p) f -> p g f", p=128
                        )
                        nc.sync.dma_start(
                            dst,
                            rowbuf0[:, 0 : gcnt * C].rearrange(
                                "p (g f) -> p g f", f=C
                            ),
                        )
                    t += gcnt
                if ntail:
                    rowbuf0 = wk.tile([128, GW * C], BF16, tag="rowbuf0",
                                      name="rowbuf0")
                    pt = pst.tile([128, 128], BF16, tag="t", name="pt")
                    nc.tensor.transpose(
                        pt[0:ntail, :], x0[:, nfull * 128 : n], id_bf[:]
                    )
                    nc.scalar.copy(rowbuf0[0:ntail, 0:C], pt[0:ntail, :])
                    nc.sync.dma_start(x_table[0][nfull * 128 : n, :],
                                      rowbuf0[0:ntail, 0:C])

            # =================================================================
            # GIN layers (x2): edge aggregation + own-only dense MLP
            # =================================================================
            def gin_layer(li, sp, out_pool):
                h = sp.tile([128, own], BF16, tag=f"h_gin{li}", name=f"h_gin{li}")
                with (
                    tc.tile_pool(name=f"gin_e{li}", bufs=2) as ep,
                    tc.tile_pool(name=f"gin_p{li}", bufs=2, space="PSUM") as pp,
                ):
                    for b in range(nblk):
                        gt = ep.tile([128, cpb, C], BF16, tag="gt", name="gt",
                                     bufs=2)
                        gather_block(gt, x_table[li], idx_src, b, C)
                        pa = pp.tile([128, 128], F32, tag="agg", name="agg")
                        for i in range(cpb):
                            col = b * cpb + i
                            s = ep.tile([128, 128], BF16, tag="sel",
                                        name="sel", bufs=3)
                            nc.vector.tensor_scalar(
                                s[:], iota[:], dstw[:, col : col + 1],
                                None, mybir.AluOpType.is_equal,
                            )
                            nc.tensor.matmul(
                                pa[:], gt[:, i, :], s[:],
                                start=(i == 0), stop=(i == cpb - 1),
                            )
                        nb = ntile_own[b]
                        nc.scalar.copy(
                            h[:, b * 128 : b * 128 + nb], pa[:, 0:nb]
                        )
                # dense: y = h @ W1; BN(global) + relu; x = h2 @ W2 + b2
                y = dense_own([w1[:]], [lambda o0, w_: h[:, o0 : o0 + w_]],
                              f"y_gin{li}", BF16, sp)
                ssum, ssq = stats_of(y, own)
                statloc = wk.tile([128, 2], F32, tag="statloc", name="statloc")
                nc.vector.tensor_copy(statloc[:, 0:1], ssum[:])
                nc.vector.tensor_copy(statloc[:, 1:2], ssq[:])
                nc.sync.dma_start(cc_stat_in[:, :], statloc[:])
                nc.gpsimd.collective_compute(
                    "AllReduce", mybir.AluOpType.add, groups,
                    [cc_stat_in[:, :].opt()], [cc_stat_out[li][:, :].opt()],
                )
                statglob = wk.tile([128, 2], F32, tag="statglob", name="statglob")
                nc.sync.dma_start(statglob[:], cc_stat_out[li][:, :])
                k, bcol = bn_cols_from_stats(
                    statglob[:, 0:1], statglob[:, 1:2], bn_gin, n
                )
                h2 = sp.tile([128, own], BF16, tag=f"h2_gin{li}", name=f"h2_gin{li}")
                nc.scalar.activation(
                    h2[:], y[:], mybir.ActivationFunctionType.Relu,
                    bias=bcol[:], scale=k[:],
                )
                x = dense_own(
                    [w2[:]], [lambda o0, w_: h2[:, o0 : o0 + w_]],
                    f"x_gin{li}", BF16, out_pool,
                    bias=b2c[:], act=mybir.ActivationFunctionType.Identity,
                )
                if li == 0:
                    write_rows(
                        [(0, C, lambda t, nt: x[:, t * 128 : t * 128 + nt])],
                        cc_rows_x, C, BF16,
                    )
                    nc.gpsimd.collective_compute(
                        "AllGather", mybir.AluOpType.bypass, groups,
                        [cc_rows_x[:, :].opt()], [x_table[1][:, :].opt()],
                    )
                return x

            if phase_limit >= 1:
                with tc.tile_pool(name="g0", bufs=1) as g0p:
                    gin_layer(0, g0p, g0p)

            zb = st.tile([128, own], BF16, tag="zb", name="zb")
            with tc.tile_pool(name="g1", bufs=1) as g1p:
                if phase_limit < 2:
                    nc.vector.memset(zb[:], 0.0)
                    x2 = None
                else:
                    x2 = gin_layer(1, g1p, g1p)

                # =============================================================
                # VAE heads (own only)
                # =============================================================
                if phase_limit < 3:
                    zt = wk.tile([128, 512], F32, tag="zf", name="zf")
                    nc.vector.memset(zt[:], 0.0)
                    for nm in ("mu_s", "logvar_s", "zin_s"):
                        for t in range(nblk):
                            nt = ntile_own[t]
                            nc.sync.dma_start(
                                outs[nm][t * 128 : t * 128 + nt, :],
                                zt[0:nt, 0:C])
                    mu = None
                else:
                    mu = dense_own([w_mu[:]], [lambda o0, w_: x2[:, o0 : o0 + w_]],
                               "mu", F32, g1p, bias=bmuc[:],
                               act=mybir.ActivationFunctionType.Identity)
                if phase_limit >= 3:
                    lv = dense_own([w_var[:]],
                                   [lambda o0, w_: x2[:, o0 : o0 + w_]],
                                   "lv", F32, g1p, bias=bvarc[:],
                                   act=mybir.ActivationFunctionType.Identity)
                    eh = g1p.tile([128, own], F32, tag="eh", name="eh")
                    nc.scalar.activation(
                        eh[:], lv[:], mybir.ActivationFunctionType.Exp, scale=0.5
                    )
                    z = g1p.tile([128, own], F32, tag="z", name="z")
                    nc.vector.tensor_mul(z[:], epsT[:], eh[:])
                    nc.vector.tensor_add(z[:], z[:], mu[:])
                    nc.vector.tensor_copy(zb[:], z[:])
                    write_rows(
                        [(0, C, lambda t, nt: mu[:, t * 128 : t * 128 + nt])],
                        outs["mu_s"], C, F32)
                    write_rows(
                        [(0, C, lambda t, nt: lv[:, t * 128 : t * 128 + nt])],
                        outs["logvar_s"], C, F32)
                    write_rows(
                        [(0, C, lambda t, nt: z[:, t * 128 : t * 128 + nt])],
                        outs["zin_s"], C, F32)

            # =================================================================
            # GAT layers (x2)
            # =================================================================
            def gat_tables(li, act):
                """act [128, own] bf16 -> big_table[li] (AllGather), ad_local."""
                with tc.tile_pool(name=f"gtab{li}", bufs=1) as tp:
                    hh = [
                        dense_own(
                            [w_gat[:, hd * C : (hd + 1) * C]],
                            [lambda o0, w_: act[:, o0 : o0 + w_]],
                            f"hh{hd}_l{li}", BF16, tp,
                        )
                        for hd in range(H)
                    ]
                    # a[kind][hd]: [1, own] row, kind 0 = a_src, 1 = a_dst
                    arow = [[None, None], [None, None]]
                    for hd in range(H):
                        for kind in range(2):
                            t_ = tp.tile([1, own], BF16, tag=f"a{kind}{hd}_l{li}",
                                         name=f"a{kind}{hd}_l{li}")
                            for (o0, w_) in dts:
                                pt = psd.tile([1, 512], F32, tag="d", name="pd")
                                nc.tensor.matmul(
                                    pt[0:1, 0:w_], attp[hd][:, kind : kind + 1],
                                    hh[hd][:, o0 : o0 + w_], start=True, stop=True,
                                )
                                nc.scalar.copy(t_[0:1, o0 : o0 + w_], pt[0:1, 0:w_])
                            arow[kind][hd] = t_
                    write_rows(
                        [
                            (0, C, lambda t, nt: hh[0][:, t * 128 : t * 128 + nt]),
                            (C + 1, C,
                             lambda t, nt: hh[1][:, t * 128 : t * 128 + nt]),
                            (2 * C + 2, 1,
                             lambda t, nt: arow[0][0][0:1, t * 128 : t * 128 + nt]),
                            (2 * C + 3, 1,
                             lambda t, nt: arow[0][1][0:1, t * 128 : t * 128 + nt]),
                        ],
                        cc_rows_big, bigw, BF16, fill_pad=True,
                        const_cols=[(C, ones_col_bf), (2 * C + 1, ones_col_bf)],
                    )
                    write_rows(
                        [
                            (0, 1,
                             lambda t, nt: arow[1][0][0:1, t * 128 : t * 128 + nt]),
                            (1, 1,
                             lambda t, nt: arow[1][1][0:1, t * 128 : t * 128 + nt]),
                        ],
                        ad_local[li], 128, BF16, fill_pad=True,
                    )
                nc.gpsimd.collective_compute(
                    "AllGather", mybir.AluOpType.bypass, groups,
                    [cc_rows_big[:, :].opt()], [big_table[li][:, :].opt()],
                )

            def gat_edge_dec(li, out_pool, out_dt):
                """Edge softmax + message aggregation + decoder matmul.

                Dst-major: pm_h [dst, hh_h | den_h] accumulates messages and
                the softmax denominator in one matmul chain per head; the
                division becomes a per-partition scale on evacuation.
                """
                SG = 8
                with (
                    tc.tile_pool(name=f"gat_s{li}", bufs=1) as gsp,
                    tc.tile_pool(name=f"gat_e{li}", bufs=2) as ep,
                    tc.tile_pool(name=f"gat_p{li}", bufs=2, space="PSUM") as pp,
                ):
                    on = [gsp.tile([128, own], BF16, tag=f"on{hd}", name=f"on{hd}")
                          for hd in range(H)]
                    for b in range(nblk):
                        gt = ep.tile([128, cpb, bigw], BF16, tag="gt",
                                     name="gt", bufs=2)
                        gather_block(gt, big_table[li], idx_src, b, bigw)
                        adg = ep.tile([128, cpb, 128], BF16, tag="adg",
                                      name="adg", bufs=2)
                        gather_block(adg, ad_local[li], idx_dstl, b, 128)
                        pm = [pp.tile([128, 129], F32, tag=f"m{hd}",
                                      name=f"m{hd}") for hd in range(H)]
                        el = ep.tile([128, cpb, 2], F32, tag="el", name="el",
                                     bufs=2)
                        for g0 in range(0, cpb, SG):
                            ng = min(SG, cpb - g0)
                            lt = ep.tile([128, SG, 2], BF16, tag="lt",
                                         name="lt", bufs=3)
                            nc.vector.tensor_add(
                                lt[:, 0:ng, :],
                                gt[:, g0 : g0 + ng, 2 * C + 2 : 2 * C + 4],
                                adg[:, g0 : g0 + ng, 0:2],
                            )
                            lm = ep.tile([128, SG, 2], BF16, tag="lm",
                                         name="lm", bufs=3)
                            nc.vector.scalar_tensor_tensor(
                                lm[:, 0:ng, :], lt[:, 0:ng, :], 0.2,
                                lt[:, 0:ng, :],
                                mybir.AluOpType.mult, mybir.AluOpType.max,
                            )
                            nc.scalar.activation(
                                el[:, g0 : g0 + ng, :], lm[:, 0:ng, :],
                                mybir.ActivationFunctionType.Exp,
                            )
                        elf = el[:, :, :].rearrange("p g t -> p (g t)")
                        for i in range(cpb):
                            col = b * cpb + i
                            se = ep.tile([128, 2, 128], BF16, tag="se",
                                         name="se", bufs=3)
                            for hd in range(H):
                                nc.vector.tensor_scalar(
                                    se[:, hd, :], iota[:],
                                    dstw[:, col : col + 1],
                                    elf[:, 2 * i + hd : 2 * i + hd + 1],
                                    mybir.AluOpType.is_equal,
                                    mybir.AluOpType.mult,
                                )
                            for hd in range(H):
                                nc.tensor.matmul(
                                    pm[hd][:, 0:129], se[:, hd, :],
                                    gt[:, i, hd * 129 : hd * 129 + 129],
                                    start=(i == 0), stop=(i == cpb - 1),
                                )
                        nb = ntile_own[b]
                        rec = wk.tile([128, H], F32, tag="rec", name="rec")
                        for hd in range(H):
                            nc.vector.reciprocal(
                                rec[0:nb, hd : hd + 1], pm[hd][0:nb, 128:129]
                            )
                        for hd in range(H):
                            ev = wk.tile([128, C], BF16, tag="ev", name="ev")
                            nc.scalar.activation(
                                ev[0:nb, :], pm[hd][0:nb, 0:128],
                                mybir.ActivationFunctionType.Identity,
                                scale=rec[0:nb, hd : hd + 1],
                            )
                            pt = pst.tile([128, 128], BF16, tag="t", name="pt")
                            nc.tensor.transpose(
                                pt[0:128, 0:nb], ev[0:nb, 0:128],
                                id_bf[0:nb, 0:nb],
                            )
                            nc.scalar.copy(
                                on[hd][:, b * 128 : b * 128 + nb], pt[:, 0:nb]
                            )
                    # decoder: zout = (out + b_gat) @ W_dec + b_dec (bias prefolded)
                    zo = dense_own(
                        [w_dec0[:], w_dec1[:]],
                        [lambda o0, w_: on[0][:, o0 : o0 + w_],
                         lambda o0, w_: on[1][:, o0 : o0 + w_]],
                        f"zo_l{li}", out_dt, out_pool,
                        bias=bdecc[:], act=mybir.ActivationFunctionType.Identity,
                    )
                return zo

            if phase_limit >= 4:
                gat_tables(0, zb)
            if phase_limit >= 5:
                zo1 = gat_edge_dec(0, st, BF16)
            if phase_limit >= 6:
                gat_tables(1, zo1)
            if phase_limit >= 7:
                with tc.tile_pool(name="zo2p", bufs=1) as zp:
                    zo2 = gat_edge_dec(1, zp, F32)
                    write_rows(
                        [(0, C, lambda t, nt: zo2[:, t * 128 : t * 128 + nt])],
                        outs["zout_s"], C, F32,
                    )
            else:
                zt0 = wk.tile([128, 512], F32, tag="zf", name="zf")
                nc.vector.memset(zt0[:], 0.0)
                for t in range(nblk):
                    nt = ntile_own[t]
                    nc.sync.dma_start(
                        outs["zout_s"][t * 128 : t * 128 + nt, :], zt0[0:nt, 0:C])
    nc.compile()
    return nc


# =====================================================================
# Host side
# =====================================================================
def host_prep(edge_index, cfg: Cfg):
    n, ncores, own, nblk = cfg.n, cfg.ncores, cfg.own, cfg.nblk
    src = np.asarray(edge_index[0], dtype=np.int64)
    dst = np.asarray(edge_index[1], dtype=np.int64)
    loop = np.arange(n, dtype=np.int64)
    src = np.concatenate([src, loop])
    dst = np.concatenate([dst, loop])
    order = np.argsort(dst, kind="stable")
    src, dst = src[order], dst[order]
    # bucket edges by (core, block)
    core_of = dst // own
    blk_in_core = (dst - core_of * own) // 128
    counts = np.zeros((ncores, nblk), dtype=np.int64)
    np.add.at(counts, (core_of, blk_in_core), 1)
    cpb = int(-(-counts.max() // 128))
    cfg.cpb = cpb
    slots = cfg.slots
    idx_src = np.zeros((ncores, slots), dtype=np.int64)
    idx_dstl = np.zeros((ncores, slots), dtype=np.int64)
    dstl = np.full((ncores, slots), -1.0, dtype=np.float32)
    # edges are sorted by dst, so per (core, block) they are contiguous
    for c in range(ncores):
        for b in range(nblk):
            b_lo = np.searchsorted(dst, c * own + b * 128)
            b_hi = np.searchsorted(dst, min(c * own + (b + 1) * 128, (c + 1) * own))
            cnt = b_hi - b_lo
            s0 = b * cpb * 128
            idx_src[c, s0 : s0 + cnt] = src[b_lo:b_hi]
            idx_dstl[c, s0 : s0 + cnt] = dst[b_lo:b_hi] - c * own
            dstl[c, s0 : s0 + cnt] = (dst[b_lo:b_hi] - c * own - b * 128).astype(
                np.float32
            )

    def wrap16(arr):
        # index k -> partition k%16 (replicated to all 8 groups), col k//16
        a = arr.reshape(-1, 16).T.astype(np.int16)  # [16, slots//16]
        return np.ascontiguousarray(np.tile(a, (8, 1)))

    per_core = {}
    for c in range(ncores):
        per_core[c] = dict(
            idx_src=wrap16(idx_src[c]),
            idx_dstl=wrap16(idx_dstl[c]),
            dstw=np.ascontiguousarray(dstl[c].reshape(slots // 128, 128).T),
        )
    return per_core


def kernel(node_features_s, edge_index_s, eps_noise,
           W_emb, b_emb, g_emb, be_emb,
           W1, b1, g1, be1, W2, b2,
           W_mu, b_mu, W_var, b_var,
           W_gat, att_src, att_dst, b_gat,
           W_dec, b_dec, _cfg=None, _nc_cache={}):
    cfg = _cfg or Cfg()
    n, own, C, H = cfg.n, cfg.own, cfg.c, cfg.h
    per_core = host_prep(edge_index_s, cfg)

    nf = np.asarray(node_features_s, dtype=np.float32)
    nfT = np.ascontiguousarray(nf.T).astype(NPBF)
    eps = np.asarray(eps_noise, dtype=np.float32)

    def colpair(a, b_):
        return np.stack([np.asarray(a, np.float32).reshape(-1),
                         np.asarray(b_, np.float32).reshape(-1)], axis=1)

    iota = np.tile(np.arange(128, dtype=np.float32), (128, 1))
    bdec_eff = (np.asarray(b_gat, np.float32) @ np.asarray(W_dec, np.float32)
                + np.asarray(b_dec, np.float32))
    shared = dict(
        nfT=nfT,
        w_emb=np.asarray(W_emb, np.float32).astype(NPBF),
        w1=np.asarray(W1, np.float32).astype(NPBF),
        w2=np.asarray(W2, np.float32).astype(NPBF),
        w_mu=np.asarray(W_mu, np.float32).astype(NPBF),
        w_var=np.asarray(W_var, np.float32).astype(NPBF),
        w_gat=np.asarray(W_gat, np.float32).astype(NPBF),
        w_dec0=np.asarray(W_dec, np.float32)[:C].astype(NPBF),
        w_dec1=np.asarray(W_dec, np.float32)[C:].astype(NPBF),
        attp0=np.stack([np.asarray(att_src, np.float32)[0],
                        np.asarray(att_dst, np.float32)[0]], axis=1).astype(NPBF),
        attp1=np.stack([np.asarray(att_src, np.float32)[1],
                        np.asarray(att_dst, np.float32)[1]], axis=1).astype(NPBF),
        bn_emb=colpair(g_emb, be_emb),
        bn_gin=colpair(g1, be1),
        b2c=np.asarray(b2, np.float32).reshape(C, 1),
        bmuc=np.asarray(b_mu, np.float32).reshape(C, 1),
        bvarc=np.asarray(b_var, np.float32).reshape(C, 1),
        bdecc=bdec_eff.reshape(C, 1),
        iota=iota.astype(NPBF),
        id_bf=np.eye(128, dtype=np.float32).astype(NPBF),
        id_f32=np.eye(128, dtype=np.float32),
        ones_col_bf=np.ones((128, 1), np.float32).astype(NPBF),
    )
    in_maps = []
    for c in range(cfg.ncores):
        m = dict(shared)
        m["epsT"] = np.ascontiguousarray(eps[c * own : (c + 1) * own].T)
        m.update(per_core[c])
        in_maps.append(m)

    pl = int(os.environ.get("KERNEL_PHASES", "99"))
    key = (cfg.n, cfg.ncores, cfg.cpb, pl)
    if key not in _nc_cache:
        _nc_cache[key] = build_program(cfg, phase_limit=pl)
    nc = _nc_cache[key]

    res = run_bass_kernel_spmd(
        nc, in_maps, core_ids=list(range(cfg.ncores)),
        trace=bool(int(os.environ.get("KERNEL_TRACE", "0"))),
    )
    results = res.results
    kernel.last_run = res

    def stitch(name):
        return np.concatenate([np.asarray(results[c][name], np.float32)
                               for c in range(cfg.ncores)], axis=0)

    return (stitch("zin_s"), stitch("zout_s"), stitch("mu_s"), stitch("logvar_s"))



# revision 13
# speedup vs baseline: 1.2691x; 1.2691x over previous
"""Trainium2 Bass kernel for nn_Denoiser (GIN-VAE encoder + GAT decoder GNN).

Distribution strategy (8 NeuronCores, SPMD single NEFF):
  - Nodes are sharded by destination ownership: core k owns nodes
    [k*2500, (k+1)*2500). Edges (with self-loops appended) are sorted by dst
    and sharded to the core owning the dst.
  - Edge aggregation uses one batched dma_gather per <=8-chunk group
    (SWDGE desc-gen on Pool is the bottleneck: ~8ns/row), followed by
    one-hot "selection" matmuls on the PE.
  - The one-hot select matrices are STATIC (edge structure) and precomputed
    on the host.  Because edges are sorted by dst, each 128-edge chunk spans
    only a narrow window of ~5-14 destinations, so the select matrices are
    stored as narrow [128, w] slices streamed from DRAM; chunk 0 is stored
    full-width to initialize the PSUM accumulation chain (start=True).
    Chunk metadata (window lo/width) must be uniform across cores (single
    SPMD program), so host metadata takes the union over the 8 cores.
  - GAT runs feature-major: pm_h[feat, dst] += hh_h_chunk.T @ se_h where
    se_h = oh_narrow * exp(leakyrelu(a_s + a_d)).  Per-edge a_d comes from a
    tiny PE matmul against the transposed one-hot (no a_d gather pass), and
    softmax denominators accumulate via two M=1 matmuls per chunk.
  - Dense per-node math is feature-major ([C partitions, nodes free]) and
    computed own-only; full tables needed for the next layer's gathers are
    materialized row-major in DRAM via PE transposes + AllGather.
  - BatchNorm (training mode, global batch stats) uses a tiny AllReduce of
    per-feature partial sums.

Host-side prep: edge sort/shard, int16 16-partition-wrapped gather indices,
narrow + transposed one-hot tables, transposed bf16/f32 inputs.
"""

import hashlib
import os
import sys

for _p in ("/opt/trn_rl_repo", "/root/.axon_site/_ro/trn_rl_repo"):
    if os.path.isdir(_p) and _p not in sys.path:
        sys.path.insert(0, _p)

from dataclasses import dataclass, field

import ml_dtypes
import numpy as np

import concourse.bacc as bacc
import concourse.bass as bass
import concourse.mybir as mybir
import concourse.tile as tile
from concourse.bass import AP
from concourse.bass_utils import run_bass_kernel_spmd

F32 = mybir.dt.float32
BF16 = mybir.dt.bfloat16
I16 = mybir.dt.int16
NPBF = ml_dtypes.bfloat16

BN_EPS = 1e-5


@dataclass
class Cfg:
    n: int = 20000        # total nodes
    ncores: int = 8
    din: int = 92
    c: int = 128          # hidden dim
    h: int = 2            # GAT heads
    cpb: int = 35         # max chunks (of 128 edge slots) per dst block
    ohw: int = 464        # one-hot table width: 128 (full chunk0) + sum(w)
    ts: int = 500         # dense free-dim tile size for own-node matmuls

    @property
    def own(self):
        return self.n // self.ncores

    @property
    def nblk(self):
        return -(-self.own // 128)

    @property
    def slots(self):
        return self.nblk * self.cpb * 128

    @property
    def bigw(self):
        # [hh0 | hh1 | a_s0 a_s1 | pad]; dma_gather rows must be a
        # multiple of 256 bytes -> 384 bf16.
        return 384


@dataclass
class Meta:
    """Uniform-across-cores chunk metadata (baked into the SPMD program)."""
    nch: list = field(default_factory=list)        # [nblk] chunks per block
    lo: list = field(default_factory=list)         # [nblk][nch] window start
    w: list = field(default_factory=list)          # [nblk][nch] window width
    off: list = field(default_factory=list)        # [nblk][nch] col in ohg

    def digest(self):
        s = repr((self.nch, self.lo, self.w, self.off)).encode()
        return hashlib.md5(s).hexdigest()


def build_program(cfg: Cfg, meta: Meta, phase_limit: int = 99) -> bass.Bass:
    nc = bacc.Bacc(
        "TRN2",
        target_bir_lowering=False,
        debug=False,
        enable_asserts=False,
        num_devices=cfg.ncores,
    )
    n, own, nblk, cpb, ts = cfg.n, cfg.own, cfg.nblk, cfg.cpb, cfg.ts
    C, H, din = cfg.c, cfg.h, cfg.din
    slots = cfg.slots
    bigw = cfg.bigw
    OHW = cfg.ohw
    groups = [list(range(cfg.ncores))]
    ntile_own = [min(128, own - t * 128) for t in range(nblk)]  # 128.. tail
    dts = [(i * ts, min(ts, own - i * ts)) for i in range(-(-own // ts))]
    # full-N dense tiles for the embedding phase
    fts = 512
    ftiles = [(i * fts, min(fts, n - i * fts)) for i in range(-(-n // fts))]
    shared_cc = bool(int(os.environ.get("KERNEL_SHARED_CC", "1")))
    cc_space = "Shared" if shared_cc else "Local"

    # ---------------- I/O ----------------
    di = {}  # dram inputs

    def inp(name, shape, dt):
        di[name] = nc.dram_tensor(name, list(shape), dt, kind="ExternalInput")
        return di[name]

    inp("nfT", [din, n], BF16)              # node features, transposed bf16
    inp("epsT", [128, own], F32)            # own eps slice, transposed
    inp("w_emb", [din, C], BF16)
    inp("w1", [C, C], BF16)
    inp("w2", [C, C], BF16)
    inp("w_mu", [C, C], BF16)
    inp("w_var", [C, C], BF16)
    inp("w_gat", [C, H * C], BF16)
    inp("w_dec0", [C, C], BF16)             # W_dec rows 0:128
    inp("w_dec1", [C, C], BF16)             # W_dec rows 128:256
    inp("attp0", [C, 2], BF16)              # [att_src[0] | att_dst[0]]
    inp("attp1", [C, 2], BF16)
    inp("bn_emb", [128, 2], F32)            # gamma | beta columns
    inp("bn_gin", [128, 2], F32)
    inp("b2c", [128, 1], F32)
    inp("bmuc", [128, 1], F32)
    inp("bvarc", [128, 1], F32)
    inp("bdecc", [128, 1], F32)             # b_gat @ W_dec + b_dec
    inp("idx_src", [128, slots // 16], I16)   # wrapped per 16p, replicated x8
    inp("ohg", [nblk * 128, OHW], BF16)     # narrow one-hots (+full chunk 0)
    inp("ohT", [nblk * 128, cpb * 128], BF16)  # transposed one-hots
    inp("id_bf", [128, 128], BF16)
    inp("id_f32", [128, 128], F32)

    outs = {}
    for nm in ("zin_s", "zout_s", "mu_s", "logvar_s"):
        outs[nm] = nc.dram_tensor(nm, [own, C], F32, kind="ExternalOutput")

    with tile.TileContext(nc) as tc:
        with (
            tc.tile_pool(name="state", bufs=1) as st,
            tc.tile_pool(name="dram", bufs=1, space="DRAM") as dr,
            tc.tile_pool(name="psum_d", bufs=2, space="PSUM") as psd,
            tc.tile_pool(name="psum_t", bufs=2, space="PSUM") as pst,
            tc.tile_pool(name="work", bufs=2) as wk,
        ):
            # ---------- load constants / inputs into SBUF ----------
            def load(name, shape, dt, pool=None):
                t = (pool or st).tile(shape, dt, tag=name, name=name)
                nc.sync.dma_start(t[:], di[name][:])
                return t

            QCH = 8  # chunks per dma_gather call (1024-descriptor ring limit)

            def gather_block(gt, table, b, elem):
                """Gather one dst block's rows in <=QCH-chunk dma_gather calls."""
                nch = meta.nch[b]
                for q0 in range(0, nch, QCH):
                    nq = min(QCH, nch - q0)
                    c0 = (b * cpb + q0) * 8
                    nc.gpsimd.dma_gather(
                        gt[:, q0 : q0 + nq, :], table[:, :],
                        idx_src[:, c0 : c0 + nq * 8],
                        nq * 128, nq * 128, elem,
                    )

            epsT = load("epsT", [128, own], F32)
            w_emb = load("w_emb", [din, C], BF16)
            w1 = load("w1", [C, C], BF16)
            w2 = load("w2", [C, C], BF16)
            w_mu = load("w_mu", [C, C], BF16)
            w_var = load("w_var", [C, C], BF16)
            w_gat = load("w_gat", [C, H * C], BF16)
            w_dec0 = load("w_dec0", [C, C], BF16)
            w_dec1 = load("w_dec1", [C, C], BF16)
            attp = [load("attp0", [C, 2], BF16), load("attp1", [C, 2], BF16)]
            bn_emb = load("bn_emb", [128, 2], F32)
            bn_gin = load("bn_gin", [128, 2], F32)
            b2c = load("b2c", [128, 1], F32)
            bmuc = load("bmuc", [128, 1], F32)
            bvarc = load("bvarc", [128, 1], F32)
            bdecc = load("bdecc", [128, 1], F32)
            idx_src = load("idx_src", [128, slots // 16], I16)
            id_bf = load("id_bf", [128, 128], BF16)
            id_f32 = load("id_f32", [128, 128], F32)

            zeros_bf = st.tile([128, 128], BF16, tag="zeros_bf", name="zeros_bf")
            nc.vector.memset(zeros_bf[:], 0.0)
            ones_row = st.tile([1, 128], BF16, tag="ones_row", name="ones_row")
            nc.vector.memset(ones_row[:], 1.0)
            ones_col = st.tile([128, 1], BF16, tag="ones_col", name="ones_col")
            nc.vector.memset(ones_col[:], 1.0)

            # persistent DRAM tables
            x_table = [
                dr.tile([n, C], BF16, tag=f"x_table{i}", name=f"x_table{i}",
                        addr_space=(cc_space if i == 1 else "Local"))
                for i in range(2)
            ]
            big_table = [
                dr.tile([n, bigw], BF16, tag=f"big_table{i}",
                        name=f"big_table{i}", addr_space=cc_space)
                for i in range(2)
            ]
            cc_rows_big = dr.tile([own, bigw], BF16, tag="cc_rows_big",
                                  name="cc_rows_big")
            cc_rows_x = dr.tile([own, C], BF16, tag="cc_rows_x",
                                name="cc_rows_x")
            cc_stat_in = dr.tile([128, 2], F32, tag="cc_stat_in",
                                 name="cc_stat_in")
            cc_stat_out = [
                dr.tile([128, 2], F32, tag=f"cc_stat_out{i}",
                        name=f"cc_stat_out{i}")
                for i in range(2)
            ]

            # ---------- helpers ----------
            def bn_cols_from_stats(ssum, ssq, gamma_beta, count):
                """Return (k, b) [128,1] f32 columns: y -> y*k + b."""
                mean = wk.tile([128, 1], F32, tag="bn_mean", name="bn_mean")
                nc.vector.tensor_scalar_mul(mean[:], ssum, 1.0 / count)
                ex2 = wk.tile([128, 1], F32, tag="bn_ex2", name="bn_ex2")
                nc.vector.tensor_scalar_mul(ex2[:], ssq, 1.0 / count)
                m2 = wk.tile([128, 1], F32, tag="bn_m2", name="bn_m2")
                nc.vector.tensor_mul(m2[:], mean[:], mean[:])
                var = wk.tile([128, 1], F32, tag="bn_var", name="bn_var")
                nc.vector.tensor_sub(var[:], ex2[:], m2[:])
                nc.vector.tensor_scalar_add(var[:], var[:], BN_EPS)
                inv = wk.tile([128, 1], F32, tag="bn_inv", name="bn_inv")
                nc.vector.reciprocal(inv[:], var[:])
                rs = wk.tile([128, 1], F32, tag="bn_rs", name="bn_rs")
                nc.scalar.sqrt(rs[:], inv[:])
                k = wk.tile([128, 1], F32, tag="bn_k", name="bn_k")
                nc.vector.tensor_mul(k[:], rs[:], gamma_beta[:, 0:1])
                mk = wk.tile([128, 1], F32, tag="bn_mk", name="bn_mk")
                nc.vector.tensor_mul(mk[:], mean[:], k[:])
                b = wk.tile([128, 1], F32, tag="bn_b", name="bn_b")
                nc.vector.tensor_sub(b[:], gamma_beta[:, 1:2], mk[:])
                return k, b

            def stats_of(ytile, width):
                """Local per-feature sum and sum-of-squares of y [128,width]."""
                ssum = wk.tile([128, 1], F32, tag="st_ssum", name="st_ssum")
                nc.vector.tensor_reduce(
                    ssum[:], ytile[:, 0:width], axis=mybir.AxisListType.X,
                    op=mybir.AluOpType.add,
                )
                sq = wk.tile([128, len(dts)], F32, tag="st_sq", name="st_sq")
                for i, (o0, w_) in enumerate(dts):
                    scr = wk.tile([128, fts], BF16, tag="scr0", name="scr0")
                    nc.scalar.activation(
                        scr[:, 0:w_], ytile[:, o0 : o0 + w_],
                        mybir.ActivationFunctionType.Square,
                        accum_out=sq[:, i : i + 1],
                    )
                ssq = wk.tile([128, 1], F32, tag="st_ssq", name="st_ssq")
                nc.vector.tensor_reduce(
                    ssq[:], sq[:], axis=mybir.AxisListType.X, op=mybir.AluOpType.add
                )
                return ssum, ssq

            GW = 8  # node tiles per batched row-write DMA

            def write_rows(cols, dst_dram, width, src_dt, grp=GW,
                           fill_pad=False, const_cols=()):
                """Transpose f-major own tiles into row-major dst_dram [own,width].

                cols: list of (col_off, ncol, get_ap(t, nt) -> [ncol-part, nt] AP).
                const_cols: list of (col_off, [128,1] const tile) copied as-is.
                Full 128-node tiles are batched grp-at-a-time into one DMA; the
                tail tile (own % 128) gets its own DMA.
                """
                full = own // 128
                tail = own % 128
                ident_t = id_bf if src_dt == BF16 else id_f32

                def rowbuf_tile():
                    rb = wk.tile([128, grp * width], src_dt,
                                 tag=f"rows{width}_{src_dt}",
                                 name=f"rows{width}_{src_dt}")
                    if fill_pad:
                        nc.vector.memset(rb[:], 0.0)
                    return rb

                def do_tile(rowbuf, t, nt, g):
                    for (co, ncol, get) in cols:
                        src = get(t, nt)
                        pt = pst.tile([128, 128], src_dt, tag="t", name="pt")
                        nc.tensor.transpose(
                            pt[0:nt, 0:ncol], src, ident_t[0:ncol, 0:ncol]
                        )
                        nc.scalar.copy(
                            rowbuf[0:nt, g * width + co : g * width + co + ncol],
                            pt[0:nt, 0:ncol],
                        )
                    for (co, ctile) in const_cols:
                        nc.scalar.copy(
                            rowbuf[0:nt, g * width + co : g * width + co + 1],
                            ctile[0:nt, 0:1],
                        )

                t = 0
                while t < full:
                    gcnt = min(grp, full - t)
                    rowbuf = rowbuf_tile()
                    for g in range(gcnt):
                        do_tile(rowbuf, t + g, 128, g)
                    if gcnt == 1:
                        nc.sync.dma_start(
                            dst_dram[t * 128 : (t + 1) * 128, :],
                            rowbuf[:, 0:width],
                        )
                    else:
                        dst = dst_dram[t * 128 : (t + gcnt) * 128, :].rearrange(
                            "(g p) f -> p g f", p=128
                        )
                        nc.sync.dma_start(
                            dst,
                            rowbuf[:, 0 : gcnt * width].rearrange(
                                "p (g f) -> p g f", f=width
                            ),
                        )
                    t += gcnt
                if tail:
                    rowbuf = rowbuf_tile()
                    do_tile(rowbuf, full, tail, 0)
                    nc.sync.dma_start(
                        dst_dram[full * 128 : own, :], rowbuf[0:tail, 0:width]
                    )

            def dense_own(lhsTs, rhs_fn, out_tag, out_dt, pool, bias=None,
                          act=None, scale=None):
                """out[128, own] = act(scale * sum_i lhsTs[i].T @ rhs_i + bias)."""
                o = pool.tile([128, own], out_dt, tag=out_tag, name=out_tag)
                if act is not None:
                    func = act
                elif bias is None and scale is None:
                    func = mybir.ActivationFunctionType.Copy
                else:
                    func = mybir.ActivationFunctionType.Identity
                for (o0, w_) in dts:
                    pt = psd.tile([128, 512], F32, tag="d", name="pd")
                    for i, (lt, rf) in enumerate(zip(lhsTs, rhs_fn)):
                        nc.tensor.matmul(
                            pt[:, 0:w_], lt, rf(o0, w_),
                            start=(i == 0), stop=(i == len(lhsTs) - 1),
                        )
                    kw = {}
                    if bias is not None:
                        kw["bias"] = bias
                    if scale is not None:
                        kw["scale"] = scale
                    nc.scalar.activation(
                        o[:, o0 : o0 + w_], pt[:, 0:w_], func, **kw,
                    )
                return o

            # =================================================================
            # Phase 0: embedding (full N, redundant on every core)
            # =================================================================
            with tc.tile_pool(name="p0", bufs=1) as p0:
                y0 = p0.tile([128, n], BF16, tag="y0", name="y0")
                for (o0, w_) in ftiles:
                    nft = wk.tile([din, fts], BF16, tag="nft", name="nft")
                    nc.sync.dma_start(nft[:, 0:w_], di["nfT"][:, o0 : o0 + w_])
                    pt = psd.tile([128, 512], F32, tag="d", name="pd")
                    nc.tensor.matmul(
                        pt[:, 0:w_], w_emb[:], nft[:, 0:w_],
                        start=True, stop=True,
                    )
                    nc.scalar.copy(y0[:, o0 : o0 + w_], pt[:, 0:w_])
                ssum0 = wk.tile([128, 1], F32, tag="ssum0", name="ssum0")
                nc.vector.tensor_reduce(
                    ssum0[:], y0[:], axis=mybir.AxisListType.X,
                    op=mybir.AluOpType.add,
                )
                sqcols = wk.tile([128, len(ftiles)], F32, tag="sqcols", name="sqcols")
                scr0 = wk.tile([128, fts], BF16, tag="scr0", name="scr0")
                for i, (o0, w_) in enumerate(ftiles):
                    nc.scalar.activation(
                        scr0[:, 0:w_], y0[:, o0 : o0 + w_],
                        mybir.ActivationFunctionType.Square,
                        accum_out=sqcols[:, i : i + 1],
                    )
                ssq0 = wk.tile([128, 1], F32, tag="ssq0", name="ssq0")
                nc.vector.tensor_reduce(
                    ssq0[:], sqcols[:], axis=mybir.AxisListType.X,
                    op=mybir.AluOpType.add,
                )
                k0, b0 = bn_cols_from_stats(ssum0[:], ssq0[:], bn_emb, n)
                x0 = p0.tile([128, n], BF16, tag="x0", name="x0")
                nc.scalar.activation(
                    x0[:], y0[:], mybir.ActivationFunctionType.Relu,
                    bias=b0[:], scale=k0[:],
                )
                # write full x0 table (row-major, local; no collective needed)
                nfull = n // 128
                ntail = n % 128
                t = 0
                while t < nfull:
                    gcnt = min(GW, nfull - t)
                    rowbuf0 = wk.tile([128, GW * C], BF16, tag="rowbuf0",
                                      name="rowbuf0")
                    for g in range(gcnt):
                        pt = pst.tile([128, 128], BF16, tag="t", name="pt")
                        nc.tensor.transpose(
                            pt[:], x0[:, (t + g) * 128 : (t + g + 1) * 128],
                            id_bf[:],
                        )
                        nc.scalar.copy(rowbuf0[:, g * C : (g + 1) * C], pt[:])
                    if gcnt == 1:
                        nc.sync.dma_start(
                            x_table[0][t * 128 : (t + 1) * 128, :], rowbuf0[:, 0:C]
                        )
                    else:
                        dst = x_table[0][t * 128 : (t + gcnt) * 128, :].rearrange(
                            "(g p) f -> p g f", p=128
                        )
                        nc.sync.dma_start(
                            dst,
                            rowbuf0[:, 0 : gcnt * C].rearrange(
                                "p (g f) -> p g f", f=C
                            ),
                        )
                    t += gcnt
                if ntail:
                    rowbuf0 = wk.tile([128, GW * C], BF16, tag="rowbuf0",
                                      name="rowbuf0")
                    pt = pst.tile([128, 128], BF16, tag="t", name="pt")
                    nc.tensor.transpose(
                        pt[0:ntail, :], x0[:, nfull * 128 : n], id_bf[:]
                    )
                    nc.scalar.copy(rowbuf0[0:ntail, 0:C], pt[0:ntail, :])
                    nc.sync.dma_start(x_table[0][nfull * 128 : n, :],
                                      rowbuf0[0:ntail, 0:C])

            # =================================================================
            # GIN layers (x2): edge aggregation + own-only dense MLP
            # =================================================================
            def gin_layer(li, sp, out_pool):
                h = sp.tile([128, own], BF16, tag=f"h_gin{li}", name=f"h_gin{li}")
                with (
                    tc.tile_pool(name=f"gin_e{li}", bufs=2) as ep,
                    tc.tile_pool(name=f"gin_p{li}", bufs=2, space="PSUM") as pp,
                ):
                    for b in range(nblk):
                        nch = meta.nch[b]
                        gt = ep.tile([128, cpb, C], BF16, tag="gt", name="gt",
                                     bufs=2)
                        gather_block(gt, x_table[li], b, C)
                        ohgb = ep.tile([128, OHW], BF16, tag="ohgb",
                                       name="ohgb", bufs=2)
                        nc.sync.dma_start(
                            ohgb[:], di["ohg"][b * 128 : (b + 1) * 128, :]
                        )
                        pa = pp.tile([128, 128], F32, tag="agg", name="agg")
                        # chunk 0: full-width one-hot initializes the bank
                        nc.tensor.matmul(
                            pa[:, 0:128], gt[:, 0, :], ohgb[:, 0:128],
                            start=True, stop=(nch == 1),
                        )
                        for i in range(1, nch):
                            lo, w_, off = meta.lo[b][i], meta.w[b][i], meta.off[b][i]
                            nc.tensor.matmul(
                                pa[:, lo : lo + w_], gt[:, i, :],
                                ohgb[:, off : off + w_],
                                start=False, stop=(i == nch - 1),
                            )
                        nb = ntile_own[b]
                        nc.scalar.copy(
                            h[:, b * 128 : b * 128 + nb], pa[:, 0:nb]
                        )
                # dense: y = h @ W1; BN(global) + relu; x = h2 @ W2 + b2
                y = dense_own([w1[:]], [lambda o0, w_: h[:, o0 : o0 + w_]],
                              f"y_gin{li}", BF16, sp)
                ssum, ssq = stats_of(y, own)
                statloc = wk.tile([128, 2], F32, tag="statloc", name="statloc")
                nc.vector.tensor_copy(statloc[:, 0:1], ssum[:])
                nc.vector.tensor_copy(statloc[:, 1:2], ssq[:])
                nc.sync.dma_start(cc_stat_in[:, :], statloc[:])
                nc.gpsimd.collective_compute(
                    "AllReduce", mybir.AluOpType.add, groups,
                    [cc_stat_in[:, :].opt()], [cc_stat_out[li][:, :].opt()],
                )
                statglob = wk.tile([128, 2], F32, tag="statglob", name="statglob")
                nc.sync.dma_start(statglob[:], cc_stat_out[li][:, :])
                k, bcol = bn_cols_from_stats(
                    statglob[:, 0:1], statglob[:, 1:2], bn_gin, n
                )
                h2 = sp.tile([128, own], BF16, tag=f"h2_gin{li}", name=f"h2_gin{li}")
                nc.scalar.activation(
                    h2[:], y[:], mybir.ActivationFunctionType.Relu,
                    bias=bcol[:], scale=k[:],
                )
                x = dense_own(
                    [w2[:]], [lambda o0, w_: h2[:, o0 : o0 + w_]],
                    f"x_gin{li}", BF16, out_pool,
                    bias=b2c[:], act=mybir.ActivationFunctionType.Identity,
                )
                if li == 0:
                    write_rows(
                        [(0, C, lambda t, nt: x[:, t * 128 : t * 128 + nt])],
                        cc_rows_x, C, BF16,
                    )
                    nc.gpsimd.collective_compute(
                        "AllGather", mybir.AluOpType.bypass, groups,
                        [cc_rows_x[:, :].opt()], [x_table[1][:, :].opt()],
                    )
                return x

            if phase_limit >= 1:
                with tc.tile_pool(name="g0", bufs=1) as g0p:
                    gin_layer(0, g0p, g0p)

            zb = st.tile([128, own], BF16, tag="zb", name="zb")
            with tc.tile_pool(name="g1", bufs=1) as g1p:
                if phase_limit < 2:
                    nc.vector.memset(zb[:], 0.0)
                    x2 = None
                else:
                    x2 = gin_layer(1, g1p, g1p)

                # =============================================================
                # VAE heads (own only)
                # =============================================================
                if phase_limit < 3:
                    zt = wk.tile([128, 512], F32, tag="zf", name="zf")
                    nc.vector.memset(zt[:], 0.0)
                    for nm in ("mu_s", "logvar_s", "zin_s"):
                        for t in range(nblk):
                            nt = ntile_own[t]
                            nc.sync.dma_start(
                                outs[nm][t * 128 : t * 128 + nt, :],
                                zt[0:nt, 0:C])
                    mu = None
                else:
                    mu = dense_own([w_mu[:]], [lambda o0, w_: x2[:, o0 : o0 + w_]],
                               "mu", F32, g1p, bias=bmuc[:],
                               act=mybir.ActivationFunctionType.Identity)
                if phase_limit >= 3:
                    lv = dense_own([w_var[:]],
                                   [lambda o0, w_: x2[:, o0 : o0 + w_]],
                                   "lv", F32, g1p, bias=bvarc[:],
                                   act=mybir.ActivationFunctionType.Identity)
                    eh = g1p.tile([128, own], F32, tag="eh", name="eh")
                    nc.scalar.activation(
                        eh[:], lv[:], mybir.ActivationFunctionType.Exp, scale=0.5
                    )
                    z = g1p.tile([128, own], F32, tag="z", name="z")
                    nc.vector.tensor_mul(z[:], epsT[:], eh[:])
                    nc.vector.tensor_add(z[:], z[:], mu[:])
                    nc.vector.tensor_copy(zb[:], z[:])
                    write_rows(
                        [(0, C, lambda t, nt: mu[:, t * 128 : t * 128 + nt])],
                        outs["mu_s"], C, F32)
                    write_rows(
                        [(0, C, lambda t, nt: lv[:, t * 128 : t * 128 + nt])],
                        outs["logvar_s"], C, F32)
                    write_rows(
                        [(0, C, lambda t, nt: z[:, t * 128 : t * 128 + nt])],
                        outs["zin_s"], C, F32)

            # =================================================================
            # GAT layers (x2)
            # =================================================================
            adv_pool = st  # advb tiles persist across the layer

            def gat_tables(li, act):
                """act [128, own] bf16 -> big_table[li] (AllGather), advb."""
                advb = adv_pool.tile([128, 2 * nblk], BF16, tag=f"advb{li}",
                                     name=f"advb{li}")
                nc.vector.memset(advb[:], 0.0)
                with tc.tile_pool(name=f"gtab{li}", bufs=1) as tp:
                    hh = [
                        dense_own(
                            [w_gat[:, hd * C : (hd + 1) * C]],
                            [lambda o0, w_: act[:, o0 : o0 + w_]],
                            f"hh{hd}_l{li}", BF16, tp,
                        )
                        for hd in range(H)
                    ]
                    # a[kind][hd]: [1, own] row, kind 0 = a_src, 1 = a_dst
                    arow = [[None, None], [None, None]]
                    for hd in range(H):
                        for kind in range(2):
                            t_ = tp.tile([1, own], BF16, tag=f"a{kind}{hd}_l{li}",
                                         name=f"a{kind}{hd}_l{li}")
                            for (o0, w_) in dts:
                                pt = psd.tile([1, 512], F32, tag="d", name="pd")
                                nc.tensor.matmul(
                                    pt[0:1, 0:w_], attp[hd][:, kind : kind + 1],
                                    hh[hd][:, o0 : o0 + w_], start=True, stop=True,
                                )
                                nc.scalar.copy(t_[0:1, o0 : o0 + w_], pt[0:1, 0:w_])
                            arow[kind][hd] = t_
                    write_rows(
                        [
                            (0, C, lambda t, nt: hh[0][:, t * 128 : t * 128 + nt]),
                            (C, C, lambda t, nt: hh[1][:, t * 128 : t * 128 + nt]),
                            (2 * C, 1,
                             lambda t, nt: arow[0][0][0:1, t * 128 : t * 128 + nt]),
                            (2 * C + 1, 1,
                             lambda t, nt: arow[0][1][0:1, t * 128 : t * 128 + nt]),
                        ],
                        cc_rows_big, bigw, BF16, fill_pad=True,
                    )
                    # advb[:, 2b+hd] = a_dst values for block b, head hd
                    for b in range(nblk):
                        nb = ntile_own[b]
                        for hd in range(H):
                            pt = pst.tile([128, 2], BF16, tag="t", name="tadv")
                            nc.tensor.transpose(
                                pt[0:nb, 0:1],
                                arow[1][hd][0:1, b * 128 : b * 128 + nb],
                                id_bf[0:1, 0:1],
                            )
                            nc.scalar.copy(
                                advb[0:nb, 2 * b + hd : 2 * b + hd + 1],
                                pt[0:nb, 0:1],
                            )
                nc.gpsimd.collective_compute(
                    "AllGather", mybir.AluOpType.bypass, groups,
                    [cc_rows_big[:, :].opt()], [big_table[li][:, :].opt()],
                )
                return advb

            def gat_edge_dec(li, advb, out_pool, out_dt):
                """Edge softmax + message aggregation + decoder matmul.

                Feature-major: pm_h[feat, dst] accumulates hh^T @ se per chunk;
                denominators accumulate via two M=1 matmuls; the per-dst
                normalization is a broadcast-row multiply on evacuation.
                """
                SG = 8
                with (
                    tc.tile_pool(name=f"gat_s{li}", bufs=1) as gsp,
                    tc.tile_pool(name=f"gat_e{li}", bufs=2) as ep,
                    tc.tile_pool(name=f"gat_p{li}", bufs=1, space="PSUM") as pp,
                ):
                    on = [gsp.tile([128, own], BF16, tag=f"on{hd}", name=f"on{hd}")
                          for hd in range(H)]
                    for b in range(nblk):
                        nch = meta.nch[b]
                        nb = ntile_own[b]
                        gt = ep.tile([128, cpb, bigw], BF16, tag="gt",
                                     name="gt", bufs=2)
                        gather_block(gt, big_table[li], b, bigw)
                        ohgb = ep.tile([128, OHW], BF16, tag="ohgb",
                                       name="ohgb", bufs=2)
                        nc.sync.dma_start(
                            ohgb[:], di["ohg"][b * 128 : (b + 1) * 128, :]
                        )
                        ohTb = ep.tile([128, cpb * 128], BF16, tag="ohTb",
                                       name="ohTb", bufs=2)
                        nc.sync.dma_start(
                            ohTb[:], di["ohT"][b * 128 : (b + 1) * 128, :]
                        )
                        pm = [pp.tile([128, 128], F32, tag=f"m{hd}",
                                      name=f"m{hd}") for hd in range(H)]
                        # both heads' softmax denominators side by side:
                        # head hd occupies cols [128*hd : 128*hd + 128)
                        pden = pp.tile([1, 256], F32, tag="den", name="den")
                        for hd in range(H):
                            nc.tensor.matmul(
                                pm[hd][:, 0:128], zeros_bf[:], zeros_bf[:],
                                start=True, stop=False,
                            )
                        nc.tensor.matmul(
                            pden[0:1, 0:256], zeros_bf[:, 0:1],
                            ohTb[:, 0:256], start=True, stop=False,
                        )
                        # per-edge logits -> exp, in groups of SG chunks
                        el_all = ep.tile([128, cpb, 2], F32, tag="el",
                                         name="el", bufs=2)
                        for g0 in range(0, nch, SG):
                            ng = min(SG, nch - g0)
                            adp = pp.tile([128, 2 * SG], F32, tag="adp",
                                          name="adp", bufs=1)
                            for k_ in range(ng):
                                i = g0 + k_
                                nc.tensor.matmul(
                                    adp[:, 2 * k_ : 2 * k_ + 2],
                                    ohTb[:, i * 128 : (i + 1) * 128],
                                    advb[:, 2 * b : 2 * b + 2],
                                    start=True, stop=True,
                                )
                            lt = ep.tile([128, SG, 2], F32, tag="lt",
                                         name="lt", bufs=3)
                            nc.vector.tensor_add(
                                lt[:, 0:ng, :],
                                adp[:, 0 : 2 * ng].rearrange(
                                    "p (g t) -> p g t", t=2),
                                gt[:, g0 : g0 + ng, 2 * C : 2 * C + 2],
                            )
                            lm = ep.tile([128, SG, 2], F32, tag="lm",
                                         name="lm", bufs=3)
                            nc.vector.scalar_tensor_tensor(
                                lm[:, 0:ng, :], lt[:, 0:ng, :], 0.2,
                                lt[:, 0:ng, :],
                                mybir.AluOpType.mult, mybir.AluOpType.max,
                            )
                            nc.scalar.activation(
                                el_all[:, g0 : g0 + ng, :], lm[:, 0:ng, :],
                                mybir.ActivationFunctionType.Exp,
                            )
                        # narrow scaled one-hots + aggregation matmuls
                        for i in range(nch):
                            lo, w_, off = (meta.lo[b][i], meta.w[b][i],
                                           meta.off[b][i])
                            se = ep.tile([128, 2, 16], BF16, tag="se",
                                         name="se", bufs=3)
                            for hd in range(H):
                                nc.vector.tensor_scalar(
                                    se[:, hd, 0:w_], ohgb[:, off : off + w_],
                                    el_all[:, i, hd : hd + 1], None,
                                    mybir.AluOpType.mult,
                                )
                            last = i == nch - 1
                            for hd in range(H):
                                nc.tensor.matmul(
                                    pm[hd][:, lo : lo + w_],
                                    gt[:, i, hd * C : (hd + 1) * C],
                                    se[:, hd, 0:w_],
                                    start=False, stop=last,
                                )
                            for hd in range(H):
                                nc.tensor.matmul(
                                    pden[0:1, 128 * hd + lo : 128 * hd + lo + w_],
                                    ones_col[:, 0:1], se[:, hd, 0:w_],
                                    start=False, stop=last,
                                )
                        # evacuate: on_h = pm_h * bcast(1/den_h)
                        for hd in range(H):
                            rden = wk.tile([1, 128], BF16, tag="rden",
                                           name="rden")
                            with nc.allow_low_precision(
                                reason="softmax denom reciprocal in bf16"
                            ):
                                nc.vector.reciprocal(
                                    rden[0:1, 0:nb],
                                    pden[0:1, 128 * hd : 128 * hd + nb],
                                )
                            rdb = pst.tile([128, 128], F32, tag="t",
                                           name="rdb")
                            nc.tensor.matmul(
                                rdb[:, 0:nb], ones_row[0:1, :],
                                rden[0:1, 0:nb], start=True, stop=True,
                            )
                            rdbs = wk.tile([128, 128], BF16, tag="rdbs",
                                           name="rdbs")
                            nc.scalar.copy(rdbs[:, 0:nb], rdb[:, 0:nb])
                            nc.vector.tensor_mul(
                                on[hd][:, b * 128 : b * 128 + nb],
                                pm[hd][:, 0:nb], rdbs[:, 0:nb],
                            )
                    # decoder: zout = (out + b_gat) @ W_dec + b_dec (bias prefolded)
                    zo = dense_own(
                        [w_dec0[:], w_dec1[:]],
                        [lambda o0, w_: on[0][:, o0 : o0 + w_],
                         lambda o0, w_: on[1][:, o0 : o0 + w_]],
                        f"zo_l{li}", out_dt, out_pool,
                        bias=bdecc[:], act=mybir.ActivationFunctionType.Identity,
                    )
                return zo

            if phase_limit >= 4:
                advb0 = gat_tables(0, zb)
            if phase_limit >= 5:
                zo1 = gat_edge_dec(0, advb0, st, BF16)
            if phase_limit >= 6:
                advb1 = gat_tables(1, zo1)
            if phase_limit >= 7:
                with tc.tile_pool(name="zo2p", bufs=1) as zp:
                    zo2 = gat_edge_dec(1, advb1, zp, F32)
                    write_rows(
                        [(0, C, lambda t, nt: zo2[:, t * 128 : t * 128 + nt])],
                        outs["zout_s"], C, F32,
                    )
            else:
                zt0 = wk.tile([128, 512], F32, tag="zf", name="zf")
                nc.vector.memset(zt0[:], 0.0)
                for t in range(nblk):
                    nt = ntile_own[t]
                    nc.sync.dma_start(
                        outs["zout_s"][t * 128 : t * 128 + nt, :], zt0[0:nt, 0:C])
    nc.compile()
    return nc


# =====================================================================
# Host side
# =====================================================================
def host_prep(edge_index, cfg: Cfg):
    """Sort/shard edges; build gather indices + one-hot tables + meta."""
    n, ncores, own, nblk = cfg.n, cfg.ncores, cfg.own, cfg.nblk
    src = np.asarray(edge_index[0], dtype=np.int64)
    dst = np.asarray(edge_index[1], dtype=np.int64)
    loop = np.arange(n, dtype=np.int64)
    src = np.concatenate([src, loop])
    dst = np.concatenate([dst, loop])
    order = np.argsort(dst, kind="stable")
    src, dst = src[order], dst[order]

    # per (core, block) edge ranges (edges sorted by dst => contiguous)
    rng = {}
    nch_cb = np.zeros((ncores, nblk), dtype=np.int64)
    for c in range(ncores):
        for b in range(nblk):
            lo_d = c * own + b * 128
            hi_d = min(c * own + (b + 1) * 128, (c + 1) * own)
            b_lo = int(np.searchsorted(dst, lo_d))
            b_hi = int(np.searchsorted(dst, hi_d))
            rng[(c, b)] = (b_lo, b_hi)
            nch_cb[c, b] = -(-(b_hi - b_lo) // 128)
    cpb = int(nch_cb.max())
    cfg.cpb = cpb

    # union chunk metadata across cores
    meta = Meta()
    for b in range(nblk):
        nch_u = int(nch_cb[:, b].max())
        meta.nch.append(nch_u)
        los, ws, offs = [], [], []
        off = 128
        for i in range(nch_u):
            lo_u, hi_u = 128, -1
            for c in range(ncores):
                b_lo, b_hi = rng[(c, b)]
                cnt = b_hi - b_lo
                if i * 128 >= cnt:
                    continue
                seg = dst[b_lo + i * 128 : min(b_lo + (i + 1) * 128, b_hi)]
                dl = seg - (c * own + b * 128)
                lo_u = min(lo_u, int(dl.min()))
                hi_u = max(hi_u, int(dl.max()))
            w = hi_u - lo_u + 1
            los.append(lo_u)
            ws.append(w)
            offs.append(off)
            off += w
        meta.lo.append(los)
        meta.w.append(ws)
        meta.off.append(offs)
    ohw = 128 + max(sum(ws) for ws in meta.w)
    ohw = -(-ohw // 8) * 8
    cfg.ohw = ohw
    slots = cfg.slots

    def wrap16(arr):
        # index k -> partition k%16 (replicated to all 8 groups), col k//16
        a = arr.reshape(-1, 16).T.astype(np.int16)  # [16, slots//16]
        return np.ascontiguousarray(np.tile(a, (8, 1)))

    per_core = {}
    for c in range(ncores):
        idx = np.zeros(slots, dtype=np.int64)
        ohg = np.zeros((nblk * 128, ohw), dtype=np.float32)
        ohT = np.zeros((nblk * 128, cpb * 128), dtype=np.float32)
        for b in range(nblk):
            b_lo, b_hi = rng[(c, b)]
            cnt = b_hi - b_lo
            s0 = b * cpb * 128
            idx[s0 : s0 + cnt] = src[b_lo:b_hi]
            dl = (dst[b_lo:b_hi] - (c * own + b * 128)).astype(np.int64)
            pos = np.arange(cnt, dtype=np.int64)
            ch = pos // 128          # chunk index
            p = pos % 128            # slot partition
            # full chunk-0 one-hot at cols [0:128)
            m0 = ch == 0
            ohg[b * 128 + p[m0], dl[m0]] = 1.0
            # narrow one-hots for all chunks
            lo_arr = np.asarray(meta.lo[b], dtype=np.int64)[ch]
            off_arr = np.asarray(meta.off[b], dtype=np.int64)[ch]
            ohg[b * 128 + p, off_arr + dl - lo_arr] = 1.0
            # transposed one-hots: [dst-local, chunk*128 + p]
            ohT[b * 128 + dl, ch * 128 + p] = 1.0
        per_core[c] = dict(
            idx_src=wrap16(idx),
            ohg=ohg.astype(NPBF),
            ohT=ohT.astype(NPBF),
        )
    return per_core, meta


def kernel(node_features_s, edge_index_s, eps_noise,
           W_emb, b_emb, g_emb, be_emb,
           W1, b1, g1, be1, W2, b2,
           W_mu, b_mu, W_var, b_var,
           W_gat, att_src, att_dst, b_gat,
           W_dec, b_dec, _cfg=None, _nc_cache={}):
    cfg = _cfg or Cfg()
    n, own, C, H = cfg.n, cfg.own, cfg.c, cfg.h
    per_core, meta = host_prep(edge_index_s, cfg)

    nf = np.asarray(node_features_s, dtype=np.float32)
    nfT = np.ascontiguousarray(nf.T).astype(NPBF)
    eps = np.asarray(eps_noise, dtype=np.float32)

    def colpair(a, b_):
        return np.stack([np.asarray(a, np.float32).reshape(-1),
                         np.asarray(b_, np.float32).reshape(-1)], axis=1)

    bdec_eff = (np.asarray(b_gat, np.float32) @ np.asarray(W_dec, np.float32)
                + np.asarray(b_dec, np.float32))
    shared = dict(
        nfT=nfT,
        w_emb=np.asarray(W_emb, np.float32).astype(NPBF),
        w1=np.asarray(W1, np.float32).astype(NPBF),
        w2=np.asarray(W2, np.float32).astype(NPBF),
        w_mu=np.asarray(W_mu, np.float32).astype(NPBF),
        w_var=np.asarray(W_var, np.float32).astype(NPBF),
        w_gat=np.asarray(W_gat, np.float32).astype(NPBF),
        w_dec0=np.asarray(W_dec, np.float32)[:C].astype(NPBF),
        w_dec1=np.asarray(W_dec, np.float32)[C:].astype(NPBF),
        attp0=np.stack([np.asarray(att_src, np.float32)[0],
                        np.asarray(att_dst, np.float32)[0]], axis=1).astype(NPBF),
        attp1=np.stack([np.asarray(att_src, np.float32)[1],
                        np.asarray(att_dst, np.float32)[1]], axis=1).astype(NPBF),
        bn_emb=colpair(g_emb, be_emb),
        bn_gin=colpair(g1, be1),
        b2c=np.asarray(b2, np.float32).reshape(C, 1),
        bmuc=np.asarray(b_mu, np.float32).reshape(C, 1),
        bvarc=np.asarray(b_var, np.float32).reshape(C, 1),
        bdecc=bdec_eff.reshape(C, 1),
        id_bf=np.eye(128, dtype=np.float32).astype(NPBF),
        id_f32=np.eye(128, dtype=np.float32),
    )
    in_maps = []
    for c in range(cfg.ncores):
        m = dict(shared)
        m["epsT"] = np.ascontiguousarray(eps[c * own : (c + 1) * own].T)
        m.update(per_core[c])
        in_maps.append(m)

    pl = int(os.environ.get("KERNEL_PHASES", "99"))
    key = (cfg.n, cfg.ncores, cfg.cpb, cfg.ohw, pl, meta.digest())
    if key not in _nc_cache:
        _nc_cache[key] = build_program(cfg, meta, phase_limit=pl)
    nc = _nc_cache[key]

    res = run_bass_kernel_spmd(
        nc, in_maps, core_ids=list(range(cfg.ncores)),
        trace=bool(int(os.environ.get("KERNEL_TRACE", "0"))),
    )
    results = res.results
    kernel.last_run = res

    def stitch(name):
        return np.concatenate([np.asarray(results[c][name], np.float32)
                               for c in range(cfg.ncores)], axis=0)

    return (stitch("zin_s"), stitch("zout_s"), stitch("mu_s"), stitch("logvar_s"))
